# revision 9
# baseline (speedup 1.0000x reference)
"""Cross-attention kernel for Trainium2, 8-core SPMD.

Problem (hardcoded shapes): B=4, N=4096, S=512, DIM=1024, H=16, D=64.
Sharding: data-parallel over B (4) x tensor-parallel over head-groups (2).
Each core computes 8 heads for one batch; host sums the two head-group
partial projection outputs per batch.

Per-core math (g = head group, b = batch):
  QT = qw_g.T @ x_b.T          [512, 4096]   (q-features on partitions)
  KT = kw_g.T @ ctx_b.T        [512, 512]
  V  = ctx_b @ vw_g            [512, 512]    (s on partitions)
  per head h (64 features), per 512-token chunk:
    S.T  = KT_h.T-slice @ QT_h [s=512, n]    scores transposed
    E    = exp(S.T * 0.125 + mask_bias)      one fused ACT op (mask per-partition)
    O'   = [V_h | 1].T @ E     [65, n]       row 64 = softmax denominator
    O.T  = O'[0:64] * (1/O'[64]) broadcast
  out_partial = O.T-as-lhsT @ pw_g + (host adds proj bias + partner partial)
"""
import os
import numpy as np

P = 128
B, N, S, DIM = 4, 4096, 512, 1024
HEADS, D = 16, 64
HG = 8               # heads per core
GF = HG * D          # 512 features per head-group
NCHUNK = 512
NCH = N // NCHUNK    # 8 chunks
KT_TILES = DIM // P  # 8 contraction tiles for projections
SCALE = D ** -0.5
MASK_NEG = -1e5

LAST_RESULTS = None
_CACHED_NC = None


def _build():
    import concourse.mybir as mybir
    import concourse.tile as tile
    from concourse import bacc

    f32 = mybir.dt.float32
    f32r = mybir.dt.float32r

    nc = bacc.Bacc("TRN2", target_bir_lowering=False, debug=False)

    xT = nc.dram_tensor("xT", [DIM, N], f32r, kind="ExternalInput")
    ctxT = nc.dram_tensor("ctxT", [DIM, S], f32r, kind="ExternalInput")
    qw = nc.dram_tensor("qw", [DIM, GF], f32r, kind="ExternalInput")
    kw = nc.dram_tensor("kw", [DIM, GF], f32r, kind="ExternalInput")
    vw = nc.dram_tensor("vw", [DIM, GF], f32r, kind="ExternalInput")
    pw = nc.dram_tensor("pw", [GF, DIM], f32r, kind="ExternalInput")
    qb = nc.dram_tensor("qb", [P, GF // P], f32, kind="ExternalInput")
    kb = nc.dram_tensor("kb", [P, GF // P], f32, kind="ExternalInput")
    vb = nc.dram_tensor("vb", [P, GF], f32, kind="ExternalInput")
    maskb = nc.dram_tensor("maskb", [P, S // P], f32, kind="ExternalInput")
    o = nc.dram_tensor("o", [N, DIM], f32, kind="ExternalOutput")

    JQ = GF // P        # 4 q-feature tiles
    ST = S // P         # 4 s tiles

    def r(ap):
        return ap

    with tile.TileContext(nc) as tc:
        with (
            tc.tile_pool(name="const", bufs=1) as cpool,
            tc.tile_pool(name="kv", bufs=1) as kvpool,
            tc.tile_pool(name="qt", bufs=2) as qtpool,
            tc.tile_pool(name="xq", bufs=2) as xqpool,
            tc.tile_pool(name="e", bufs=2) as epool,
            tc.tile_pool(name="ot", bufs=2) as otpool,
            tc.tile_pool(name="sm", bufs=4) as smpool,
            tc.tile_pool(name="ost", bufs=2) as ostpool,
            tc.tile_pool(name="ps_p", bufs=2, space="PSUM") as ps_p,
            tc.tile_pool(name="ps_s", bufs=3, space="PSUM") as ps_s,
            tc.tile_pool(name="ps_o", bufs=2, space="PSUM") as ps_o,
        ):
            # ---- load weights + constants ----
            qw_sb = cpool.tile([P, KT_TILES, GF], f32r)
            kw_sb = cpool.tile([P, KT_TILES, GF], f32r)
            vw_sb = cpool.tile([P, KT_TILES, GF], f32r)
            pw_sb = cpool.tile([P, GF // P, DIM], f32r)
            ctx_sb = xqpool.tile([P, KT_TILES, S], f32r, tag="xq")
            qb_sb = cpool.tile([P, JQ], f32)
            kb_sb = cpool.tile([P, JQ], f32)
            vb_sb = cpool.tile([P, GF], f32)
            mask_sb = cpool.tile([P, ST], f32)

            nc.sync.dma_start(qw_sb[:], qw.rearrange("(ko ki) m -> ki ko m", ki=P))
            nc.sync.dma_start(kw_sb[:], kw.rearrange("(ko ki) m -> ki ko m", ki=P))
            nc.sync.dma_start(vw_sb[:], vw.rearrange("(ko ki) m -> ki ko m", ki=P))
            nc.sync.dma_start(pw_sb[:], pw.rearrange("(ko ki) m -> ki ko m", ki=P))
            nc.sync.dma_start(ctx_sb[:], ctxT.rearrange("(ko ki) s -> ki ko s", ki=P))
            nc.sync.dma_start(qb_sb[:], qb[:])
            nc.sync.dma_start(kb_sb[:], kb[:])
            nc.sync.dma_start(vb_sb[:], vb[:])
            nc.sync.dma_start(mask_sb[:], maskb[:])

            # ---- KT = kw.T @ ctxT  -> [128, JQ, S] (kfeat on partitions) ----
            kt_sb = kvpool.tile([P, JQ, S], f32r)
            for jk in range(JQ):
                ps = ps_p.tile([P, S], f32, tag="proj_ps")
                for k in range(KT_TILES):
                    nc.tensor.matmul(
                        ps[:], r(kw_sb[:, k, jk * P:(jk + 1) * P]), r(ctx_sb[:, k, :]),
                        start=(k == 0), stop=(k == KT_TILES - 1))
                nc.vector.tensor_scalar_add(kt_sb[:, jk, :], ps[:], kb_sb[:, jk:jk + 1])

            # ---- V = ctx @ vw -> [128(s), ST, HG, 65] with ones column ----
            v_sb = kvpool.tile([P, ST, HG, D + 1], f32r)
            # ones column (col D) for the softmax-denominator trick; memset
            # can't encode f32r so write 1.0 = in*0 + 1 via ACT Identity
            nc.scalar.activation(
                v_sb[:, :, :, D],
                vb_sb[:, 0:ST * HG].rearrange("p (a b) -> p a b", a=ST),
                mybir.ActivationFunctionType.Identity,
                bias=1.0, scale=0.0)
            for st in range(ST):
                ps = ps_p.tile([P, GF], f32, tag="proj_ps")
                for k in range(KT_TILES):
                    nc.tensor.matmul(
                        ps[:], r(ctx_sb[:, k, st * P:(st + 1) * P]), r(vw_sb[:, k, :]),
                        start=(k == 0), stop=(k == KT_TILES - 1))
                nc.vector.tensor_add(
                    v_sb[:, st, :, 0:D],
                    ps.rearrange("p (h d) -> p h d", h=HG),
                    vb_sb.rearrange("p (h d) -> p h d", h=HG))

            # ---- main loop over token chunks ----
            for c in range(NCH):
                # QT chunk: [128, JQ, NCHUNK] (q-features on partitions)
                xq = xqpool.tile([P, KT_TILES, NCHUNK], f32r, tag="xq")
                nc.sync.dma_start(
                    xq[:],
                    xT.rearrange("(ko ki) n -> ki ko n", ki=P)[
                        :, :, c * NCHUNK:(c + 1) * NCHUNK])
                qt = qtpool.tile([P, JQ, NCHUNK], f32r, tag="qt")
                for jq in range(JQ):
                    ps = ps_p.tile([P, NCHUNK], f32, tag="proj_ps")
                    for k in range(KT_TILES):
                        nc.tensor.matmul(
                            ps[:], r(qw_sb[:, k, jq * P:(jq + 1) * P]), r(xq[:, k, :]),
                            start=(k == 0), stop=(k == KT_TILES - 1))
                    nc.vector.tensor_scalar_add(qt[:, jq, :], ps[:], qb_sb[:, jq:jq + 1])

                # attention for the 8 heads of this chunk
                ot = otpool.tile([P, JQ, NCHUNK], f32r, tag="ot")
                for h in range(HG):
                    hb = (h % 2) * 64          # partition base for this head
                    jh = h // 2
                    qt_h = qt[hb:hb + 64, jh, :]          # [64, NCHUNK]
                    e = epool.tile([P, ST, NCHUNK], f32r, tag="e")
                    for st in range(ST):
                        sps = ps_s.tile([P, NCHUNK], f32, tag="s_ps")
                        nc.tensor.matmul(
                            sps[:],
                            r(kt_sb[hb:hb + 64, jh, st * P:(st + 1) * P]),
                            r(qt_h),
                            start=True, stop=True)
                        nc.scalar.activation(
                            e[:, st, :], sps[:],
                            mybir.ActivationFunctionType.Exp,
                            bias=mask_sb[:, st:st + 1], scale=SCALE)
                    ops = ps_o.tile([D + 1, NCHUNK], f32, tag="o_ps")
                    for st in range(ST):
                        nc.tensor.matmul(
                            ops[:], r(v_sb[:, st, h, :]), r(e[:, st, :]),
                            start=(st == 0), stop=(st == ST - 1))
                    recip = smpool.tile([1, NCHUNK], f32, tag="recip")
                    nc.vector.reciprocal(recip[:], ops[D:D + 1, :])
                    rb = smpool.tile([64, NCHUNK], f32, tag="rb")
                    nc.gpsimd.partition_broadcast(rb[:], recip[:])
                    nc.vector.tensor_mul(ot[hb:hb + 64, jh, :], ops[0:D, :], rb[:])

                # out chunk: lhsT = ot tiles, rhs = pw
                for ns in range(NCHUNK // P):
                    ostage = ostpool.tile([P, DIM], f32, tag="ostage")
                    for fh in range(2):
                        ps = ps_p.tile([P, DIM // 2], f32, tag="proj_ps")
                        for j in range(JQ):
                            nc.tensor.matmul(
                                ps[:],
                                r(ot[:, j, ns * P:(ns + 1) * P]),
                                r(pw_sb[:, j, fh * 512:(fh + 1) * 512]),
                                start=(j == 0), stop=(j == JQ - 1))
                        nc.vector.tensor_copy(ostage[:, fh * 512:(fh + 1) * 512], ps[:])
                    nc.sync.dma_start(
                        o[c * NCHUNK + ns * P: c * NCHUNK + (ns + 1) * P, :],
                        ostage[:])

    nc.compile()
    return nc


def _get_nc():
    global _CACHED_NC
    if _CACHED_NC is None:
        _CACHED_NC = _build()
    return _CACHED_NC


def kernel(x, context, context_mask, q_w, q_b, kv_w, kv_b, proj_w, proj_b):
    global LAST_RESULTS
    from concourse.bass_utils import run_bass_kernel_spmd

    x = np.asarray(x, dtype=np.float32)
    context = np.asarray(context, dtype=np.float32)
    context_mask = np.asarray(context_mask)
    q_w = np.asarray(q_w, dtype=np.float32)
    q_b = np.asarray(q_b, dtype=np.float32)
    kv_w = np.asarray(kv_w, dtype=np.float32)
    kv_b = np.asarray(kv_b, dtype=np.float32)
    proj_w = np.asarray(proj_w, dtype=np.float32)
    proj_b = np.asarray(proj_b, dtype=np.float32)

    c = np.ascontiguousarray

    in_maps = []
    for dev in range(8):
        b, g = dev // 2, dev % 2
        gs = g * GF
        mask_neg = np.where(context_mask[b], np.float32(MASK_NEG), np.float32(0.0))
        in_maps.append({
            "xT": c(x[b].T),
            "ctxT": c(context[b].T),
            "qw": c(q_w[:, gs:gs + GF]),
            "kw": c(kv_w[:, gs:gs + GF]),
            "vw": c(kv_w[:, DIM + gs:DIM + gs + GF]),
            "pw": c(proj_w[gs:gs + GF, :]),
            "qb": c(q_b[gs:gs + GF].reshape(GF // P, P).T),
            "kb": c(kv_b[gs:gs + GF].reshape(GF // P, P).T),
            "vb": c(np.broadcast_to(kv_b[DIM + gs:DIM + gs + GF][None, :], (P, GF))),
            "maskb": c(mask_neg.astype(np.float32).reshape(S // P, P).T),
        })

    nc = _get_nc()
    res = run_bass_kernel_spmd(nc, in_maps, core_ids=list(range(8)))
    LAST_RESULTS = res

    out = np.empty((B, N, DIM), dtype=np.float32)
    for b in range(B):
        out[b] = res.results[2 * b]["o"] + res.results[2 * b + 1]["o"] + proj_b
    return out


# revision 18
# speedup vs baseline: 1.1250x; 1.1250x over previous
"""Cross-attention kernel for Trainium2, 8-core SPMD.

Problem (hardcoded shapes): B=4, N=4096, S=512, DIM=1024, H=16, D=64.
Sharding: data-parallel over B (4) x tensor-parallel over head-groups (2).
Each core computes 8 heads for one batch; host sums the two head-group
partial projection outputs per batch.

Per-core math (g = head group, b = batch):
  QT = qw_g.T @ x_b.T          [512, 4096]   (q-features on partitions)
  KT = kw_g.T @ ctx_b.T        [512, 512]
  V  = ctx_b @ vw_g            [512, 512]    (s on partitions)
  per head h (64 features), per 512-token chunk:
    S.T  = KT_h.T-slice @ QT_h [s=512, n]    scores transposed
    E    = exp(S.T * 0.125 + mask_bias)      one fused ACT op (mask per-partition)
    O'   = [V_h | 1].T @ E     [65, n]       row 64 = softmax denominator
    O.T  = O'[0:64] * (1/O'[64]) broadcast
  out_partial = O.T-as-lhsT @ pw_g + (host adds proj bias + partner partial)
"""
import os
import numpy as np

P = 128
B, N, S, DIM = 4, 4096, 512, 1024
HEADS, D = 16, 64
HG = 8               # heads per core
GF = HG * D          # 512 features per head-group
NCHUNK = 512
NCH = N // NCHUNK    # 8 chunks
KT_TILES = DIM // P  # 8 contraction tiles for projections
SCALE = D ** -0.5
MASK_NEG = -1e5

LAST_RESULTS = None
_CACHED_NC = None


def _build():
    import concourse.mybir as mybir
    import concourse.tile as tile
    from concourse import bacc

    f32 = mybir.dt.float32
    f32r = mybir.dt.float16  # matmul operand dtype: fp16 streams 1 col/cycle (fp32/fp32r take 2)

    nc = bacc.Bacc("TRN2", target_bir_lowering=False, debug=False)

    xT = nc.dram_tensor("xT", [DIM, N], f32r, kind="ExternalInput")
    ctxT = nc.dram_tensor("ctxT", [DIM, S], f32r, kind="ExternalInput")
    qw = nc.dram_tensor("qw", [DIM, GF], f32r, kind="ExternalInput")
    kw = nc.dram_tensor("kw", [DIM, GF], f32r, kind="ExternalInput")
    vw = nc.dram_tensor("vw", [DIM, GF], f32r, kind="ExternalInput")
    pw = nc.dram_tensor("pw", [GF, DIM], f32r, kind="ExternalInput")
    qb = nc.dram_tensor("qb", [P, GF // P], f32, kind="ExternalInput")
    kb = nc.dram_tensor("kb", [P, GF // P], f32, kind="ExternalInput")
    vbm = nc.dram_tensor("vbm", [P, S // P, GF], f32, kind="ExternalInput")
    m01 = nc.dram_tensor("m01", [P, S // P], f32, kind="ExternalInput")
    o = nc.dram_tensor("o", [N, DIM], f32, kind="ExternalOutput")

    JQ = GF // P        # 4 q-feature tiles
    ST = S // P         # 4 s tiles

    def r(ap):
        return ap

    with tile.TileContext(nc) as tc:
        with (
            tc.tile_pool(name="const", bufs=1) as cpool,
            tc.tile_pool(name="kv", bufs=1) as kvpool,
            tc.tile_pool(name="qt", bufs=2) as qtpool,
            tc.tile_pool(name="xq", bufs=2) as xqpool,
            tc.tile_pool(name="e", bufs=3) as epool,
            tc.tile_pool(name="ot", bufs=2) as otpool,
            tc.tile_pool(name="sm", bufs=4) as smpool,
            tc.tile_pool(name="ost", bufs=2) as ostpool,
            tc.tile_pool(name="ps_p", bufs=2, space="PSUM") as ps_p,
            tc.tile_pool(name="ps_s", bufs=2, space="PSUM") as ps_s,
            tc.tile_pool(name="ps_o", bufs=2, space="PSUM") as ps_o,
        ):
            # ---- load weights + constants ----
            qw_sb = cpool.tile([P, KT_TILES, GF], f32r)
            kw_sb = cpool.tile([P, KT_TILES, GF], f32r)
            vw_sb = cpool.tile([P, KT_TILES, GF], f32r)
            pw_sb = cpool.tile([P, GF // P, DIM], f32r)
            ctx_sb = xqpool.tile([P, KT_TILES, S], f32r, tag="xq")
            qb_sb = cpool.tile([P, JQ], f32)
            kb_sb = cpool.tile([P, JQ], f32)
            vbm_sb = cpool.tile([P, ST, GF], f32)
            m01_sb = cpool.tile([P, ST], f32)

            nc.sync.dma_start(qw_sb[:], qw.rearrange("(ko ki) m -> ki ko m", ki=P))
            nc.sync.dma_start(kw_sb[:], kw.rearrange("(ko ki) m -> ki ko m", ki=P))
            nc.sync.dma_start(vw_sb[:], vw.rearrange("(ko ki) m -> ki ko m", ki=P))
            nc.sync.dma_start(pw_sb[:], pw.rearrange("(ko ki) m -> ki ko m", ki=P))
            nc.sync.dma_start(ctx_sb[:], ctxT.rearrange("(ko ki) s -> ki ko s", ki=P))
            nc.sync.dma_start(qb_sb[:], qb[:])
            nc.sync.dma_start(kb_sb[:], kb[:])
            nc.sync.dma_start(vbm_sb[:], vbm[:])
            nc.sync.dma_start(m01_sb[:], m01[:])

            # ---- KT = kw.T @ ctxT  -> [128, JQ, S] (kfeat on partitions) ----
            kt_sb = kvpool.tile([P, JQ, S], f32r)
            # persistent denominator-staging tile; only partitions {0,32,64,96}
            # carry real data, the rest stay at 1.0 so the bulk reciprocal is
            # deterministic
            stage8 = kvpool.tile([P, 2, NCHUNK], f32)
            nc.vector.memset(stage8[:], 1.0)
            for jk in range(JQ):
                ps = ps_p.tile([P, S], f32, tag="proj_ps")
                for k in range(KT_TILES):
                    nc.tensor.matmul(
                        ps[:], r(kw_sb[:, k, jk * P:(jk + 1) * P]), r(ctx_sb[:, k, :]),
                        start=(k == 0), stop=(k == KT_TILES - 1))
                nc.vector.tensor_scalar_add(kt_sb[:, jk, :], ps[:], kb_sb[:, jk:jk + 1])

            # ---- V = ctx @ vw -> [128(s), ST, HG, 65] with ones column ----
            # V rows for masked s are zeroed and the denominator column (col D)
            # holds the 0/1 mask, so masked positions drop out of both the
            # numerator and denominator -- exp then needs no mask bias at all.
            v_sb = kvpool.tile([P, ST, HG, D + 1], f32r)
            for st in range(ST):
                ps = ps_p.tile([P, GF], f32, tag="proj_ps")
                for k in range(KT_TILES):
                    nc.tensor.matmul(
                        ps[:], r(ctx_sb[:, k, st * P:(st + 1) * P]), r(vw_sb[:, k, :]),
                        start=(k == 0), stop=(k == KT_TILES - 1))
                nc.vector.scalar_tensor_tensor(
                    v_sb[:, st, :, 0:D],
                    ps.rearrange("p (h d) -> p h d", h=HG),
                    m01_sb[:, st:st + 1],
                    vbm_sb[:, st, :].rearrange("p (h d) -> p h d", h=HG),
                    mybir.AluOpType.mult, mybir.AluOpType.add)
                nc.scalar.activation(
                    v_sb[:, st, :, D], vbm_sb[:, st, 0:HG],
                    mybir.ActivationFunctionType.Identity,
                    bias=m01_sb[:, st:st + 1], scale=0.0)

            # ---- main loop over token chunks ----
            for c in range(NCH):
                # QT chunk: [128, JQ, NCHUNK] (q-features on partitions)
                xq = xqpool.tile([P, KT_TILES, NCHUNK], f32r, tag="xq")
                nc.sync.dma_start(
                    xq[:],
                    xT.rearrange("(ko ki) n -> ki ko n", ki=P)[
                        :, :, c * NCHUNK:(c + 1) * NCHUNK])
                qt = qtpool.tile([P, JQ, NCHUNK], f32r, tag="qt")
                for jq in range(JQ):
                    ps = ps_p.tile([P, NCHUNK], f32, tag="proj_ps")
                    for k in range(KT_TILES):
                        nc.tensor.matmul(
                            ps[:], r(qw_sb[:, k, jq * P:(jq + 1) * P]), r(xq[:, k, :]),
                            start=(k == 0), stop=(k == KT_TILES - 1))
                    nc.vector.tensor_scalar_add(qt[:, jq, :], ps[:], qb_sb[:, jq:jq + 1])

                # attention for the 8 heads of this chunk
                ot = otpool.tile([P, JQ, NCHUNK], f32r, tag="ot")
                for h in range(HG):
                    hb = (h % 2) * 64          # partition base for this head
                    jh = h // 2
                    qt_h = qt[hb:hb + 64, jh, :]          # [64, NCHUNK]
                    e = epool.tile([P, ST, NCHUNK], f32r, tag="e")
                    for stp in range(ST // 2):
                        # two s-tiles share one 2-bank PSUM tile so the
                        # exp covers 1024 elems/lane in a single ACT op
                        sps = ps_s.tile([P, 2, NCHUNK], f32, tag="s_ps")
                        for i in range(2):
                            st = 2 * stp + i
                            nc.tensor.matmul(
                                sps[:, i, :],
                                r(kt_sb[hb:hb + 64, jh, st * P:(st + 1) * P]),
                                r(qt_h),
                                start=True, stop=True)
                        nc.scalar.activation(
                            e[:, 2 * stp:2 * stp + 2, :], sps[:],
                            mybir.ActivationFunctionType.Exp, scale=SCALE)
                    ops = ps_o.tile([D + 1, NCHUNK], f32, tag="o_ps")
                    for st in range(ST):
                        nc.tensor.matmul(
                            ops[:], r(v_sb[:, st, h, :]), r(e[:, st, :]),
                            start=(st == 0), stop=(st == ST - 1))
                    # stash denominator row + unnormalized O.T; engines alternate
                    sb_, fb_ = 32 * (h % 4), h // 4
                    if h % 2 == 0:
                        nc.vector.tensor_copy(stage8[sb_:sb_ + 1, fb_, :],
                                              ops[D:D + 1, :])
                        nc.vector.tensor_copy(ot[hb:hb + 64, jh, :], ops[0:D, :])
                    else:
                        nc.scalar.copy(stage8[sb_:sb_ + 1, fb_, :], ops[D:D + 1, :])
                        nc.scalar.copy(ot[hb:hb + 64, jh, :], ops[0:D, :])
                # one reciprocal for all 8 heads' denominators
                recip8 = smpool.tile([P, 2, NCHUNK], f32, tag="recip8")
                nc.vector.reciprocal(recip8[:], stage8[:])
                for h in range(HG):
                    hb = (h % 2) * 64
                    jh = h // 2
                    sb_, fb_ = 32 * (h % 4), h // 4
                    # partition_broadcast's gpsimd ucode only reads offset-0
                    # source APs (nonzero offsets return garbage on HW), so
                    # bounce each head's row through a fresh tile first
                    rcp = smpool.tile([1, NCHUNK], f32, tag="rcp")
                    if h % 2 == 0:
                        nc.vector.tensor_copy(rcp[:], recip8[sb_:sb_ + 1, fb_, :])
                    else:
                        nc.scalar.copy(rcp[:], recip8[sb_:sb_ + 1, fb_, :])
                    rb = smpool.tile([P, NCHUNK], f32, tag="rb")
                    nc.gpsimd.partition_broadcast(rb[:], rcp[:])
                    nc.vector.tensor_mul(ot[hb:hb + 64, jh, :],
                                         ot[hb:hb + 64, jh, :],
                                         rb[hb:hb + 64, :])

                # out chunk: lhsT = ot tiles, rhs = pw
                for ns in range(NCHUNK // P):
                    ostage = ostpool.tile([P, DIM], f32, tag="ostage")
                    for fh in range(2):
                        ps = ps_p.tile([P, DIM // 2], f32, tag="proj_ps")
                        for j in range(JQ):
                            nc.tensor.matmul(
                                ps[:],
                                r(ot[:, j, ns * P:(ns + 1) * P]),
                                r(pw_sb[:, j, fh * 512:(fh + 1) * 512]),
                                start=(j == 0), stop=(j == JQ - 1))
                        nc.vector.tensor_copy(ostage[:, fh * 512:(fh + 1) * 512], ps[:])
                    nc.sync.dma_start(
                        o[c * NCHUNK + ns * P: c * NCHUNK + (ns + 1) * P, :],
                        ostage[:])

    nc.compile()
    return nc


def _get_nc():
    global _CACHED_NC
    if _CACHED_NC is None:
        _CACHED_NC = _build()
    return _CACHED_NC


def kernel(x, context, context_mask, q_w, q_b, kv_w, kv_b, proj_w, proj_b):
    global LAST_RESULTS
    from concourse.bass_utils import run_bass_kernel_spmd

    x = np.asarray(x, dtype=np.float32)
    context = np.asarray(context, dtype=np.float32)
    context_mask = np.asarray(context_mask)
    q_w = np.asarray(q_w, dtype=np.float32)
    q_b = np.asarray(q_b, dtype=np.float32)
    kv_w = np.asarray(kv_w, dtype=np.float32)
    kv_b = np.asarray(kv_b, dtype=np.float32)
    proj_w = np.asarray(proj_w, dtype=np.float32)
    proj_b = np.asarray(proj_b, dtype=np.float32)

    c = np.ascontiguousarray

    in_maps = []
    for dev in range(8):
        b, g = dev // 2, dev % 2
        gs = g * GF
        m01_np = np.where(context_mask[b], np.float32(0.0), np.float32(1.0))
        h16 = np.float16
        in_maps.append({
            "xT": c(x[b].T.astype(h16)),
            "ctxT": c(context[b].T.astype(h16)),
            "qw": c(q_w[:, gs:gs + GF].astype(h16)),
            "kw": c(kv_w[:, gs:gs + GF].astype(h16)),
            "vw": c(kv_w[:, DIM + gs:DIM + gs + GF].astype(h16)),
            "pw": c(proj_w[gs:gs + GF, :].astype(h16)),
            "qb": c(q_b[gs:gs + GF].reshape(GF // P, P).T),
            "kb": c(kv_b[gs:gs + GF].reshape(GF // P, P).T),
            "vbm": c(m01_np.reshape(S // P, P).T[:, :, None]
                     * kv_b[DIM + gs:DIM + gs + GF][None, None, :]).astype(np.float32),
            "m01": c(m01_np.reshape(S // P, P).T),
        })

    nc = _get_nc()
    res = run_bass_kernel_spmd(nc, in_maps, core_ids=list(range(8)))
    LAST_RESULTS = res

    out = np.empty((B, N, DIM), dtype=np.float32)
    for b in range(B):
        out[b] = res.results[2 * b]["o"] + res.results[2 * b + 1]["o"] + proj_b
    return out
